# revision 7
# baseline (speedup 1.0000x reference)
"""Trainium2 Bass kernel for nn_BiLSTM_2491081031886.

Single-layer unidirectional LSTM (B=2048, T=256, F=H=128) + Linear([T*H]->1).
Data-parallel over 8 NeuronCores: each core owns a 256-row batch shard and
runs the full sequential scan locally; weights are replicated.

Per-core dataflow (all layouts [hidden/partition, batch/free]):
  - x is DMA'd in with an fp32->bf16 cast (SWDGE), then DMA-xbar-transposed
    to xT_t tiles [F=128, 256] in SBUF.
  - Per step t, gates land in PSUM as 5 blocks [i|f|o|2g|cd] x 256 cols
    (two ping-pong step buffers, bank-aligned at cols 0 and 1536).
    gates = bias (K=4 matmul vs a block-indicator) + W_ih^T.T @ xT_t (bf16)
          + W_hh2^T.T @ h_half (fp32), accumulated in PSUM.
  - One packed Sigmoid ACT op per wave reads [i,f,o,2g,cd] (640 cols) and
    writes bf16 to SBUF. tanh(g) = 2*sigmoid(2g)-1 and tanh(c) = 2*sigmoid(2c)-1
    are realized by pre-scaling weights (x2 on the g-chunk) and by storing the
    doubled cell state cd = 2c.
  - DVE cell math per wave (fused scalar_tensor_tensor ops):
      t2h = (sig2g - 0.5) * sigi          # = tanh(g)*sigi / 2
      u   = sigf * cd_prev                # PSUM read
      cd  = 2*t2h + u                     # PSUM write (next wave's sigma block)
      h_half = (sigcd - 0.5) * sigo       # = h/2; absorbed by 2x on W_hh, w_lin
  - Two interleaved batch waves (cols 0:128 / 128:256) hide the serial
    matmul->sigmoid->DVE latency chain.
  - Output head: acc[1,128] += (2*w_lin_t) as lhsT against h_half, accumulated
    in PSUM over all 256 steps; +b_lin on host.
"""

import numpy as np
import ml_dtypes

import concourse.bacc as bacc
import concourse.bass as bass
import concourse.mybir as mybir
from concourse import tile
from concourse.bass_utils import run_bass_kernel_spmd

F32 = mybir.dt.float32
BF16 = mybir.dt.bfloat16
AF = mybir.ActivationFunctionType
OP = mybir.AluOpType

B, T_FULL, F = 2048, 256, 128
H = F
NCORES = 8
BS = B // NCORES  # 256 batch rows per core
W2 = 128          # wave width (batch cols per wave)
TC = 8            # timesteps per x-ingest chunk

# PSUM column layout (fp32 words per partition, 4096 total = 8 banks x 512)
PS_BUF = (0, 1536)     # two step buffers, 3 banks apart
BLK = 256              # block width: [i|f|o|2g|cd] each 256 cols (A:0-127 B:128-255)
CD = 4 * BLK           # cd block offset within a buffer
# wlin accumulators [1, 128] each. MUST be in separate banks: a matmul with
# start=True resets has_written for the whole PSUM bank, which would wipe the
# other wave's long-lived accumulation group.
WLIN_A = 3072          # bank 6
WLIN_B = 3584          # bank 7


def build(T=T_FULL, dump=False):
    nc = bacc.Bacc("TRN2", target_bir_lowering=False, debug=False,
                   num_devices=NCORES)

    x_d = nc.dram_tensor("x", [BS, T, F], F32, kind="ExternalInput")
    whh_d = nc.dram_tensor("whh", [H, 4 * H], F32, kind="ExternalInput")
    wih_d = nc.dram_tensor("wih", [F, 4 * H], BF16, kind="ExternalInput")
    b4_d = nc.dram_tensor("b4", [4, H], BF16, kind="ExternalInput")
    e4_d = nc.dram_tensor("e4", [4, 1024], BF16, kind="ExternalInput")
    wl_d = nc.dram_tensor("wl", [H, T], F32, kind="ExternalInput")
    out_d = nc.dram_tensor("out", [BS], F32, kind="ExternalOutput")
    if dump:
        sa_d = nc.dram_tensor("sa_dump", [T, 128, 640], F32, kind="ExternalOutput")
        sb_d = nc.dram_tensor("sb_dump", [T, 128, 640], F32, kind="ExternalOutput")
        hha_d = nc.dram_tensor("hha_dump", [T, H, W2], F32, kind="ExternalOutput")
        hhb_d = nc.dram_tensor("hhb_dump", [T, H, W2], F32, kind="ExternalOutput")

    n_chunks = (T + TC - 1) // TC

    with tile.TileContext(nc) as tc:
        with (
            tc.tile_pool(name="const", bufs=1) as constp,
            tc.tile_pool(name="xstage", bufs=4) as xsp,
            tc.tile_pool(name="xT", bufs=12) as xtp,
            tc.tile_pool(name="sig", bufs=3) as sigp,
            tc.tile_pool(name="hh", bufs=3) as hhp,
            tc.tile_pool(name="tmp", bufs=2) as tmpp,
            tc.tile_pool(name="psum", bufs=1, space=bass.MemorySpace.PSUM) as psp,
        ):
            # ---- constants ----
            whh = constp.tile([H, 4 * H], F32)
            wih = constp.tile([F, 4 * H], BF16)
            b4 = constp.tile([4, H], BF16)
            e4 = constp.tile([4, 1024], BF16)
            wl = constp.tile([H, T], F32)
            nc.sync.dma_start(whh[:], whh_d.ap())
            nc.sync.dma_start(wih[:], wih_d.ap())
            nc.sync.dma_start(b4[:], b4_d.ap())
            nc.sync.dma_start(e4[:], e4_d.ap())
            nc.sync.dma_start(wl[:], wl_d.ap())

            ps = psp.tile([128, 4096], F32)
            # cd_B(-1) read by sigma-op A(0) / uB(0); cd_A(-1) read by uA(0)
            nc.vector.memset(ps[:, PS_BUF[0] + CD: PS_BUF[0] + CD + W2], 0.0)
            nc.vector.memset(ps[:, PS_BUF[1] + CD + W2: PS_BUF[1] + CD + 2 * W2], 0.0)

            # ---- x ingest: cast-DMA to bf16, then xbar-transpose per t ----
            xT = []
            xap = x_d.ap()
            for ch in range(n_chunks):
                t0 = ch * TC
                tc_n = min(TC, T - t0)
                stages = []
                for hb in range(2):
                    xs = xsp.tile([128, TC, F], BF16, tag=f"xs{hb}")
                    nc.gpsimd.dma_start(
                        xs[:, 0:tc_n, :],
                        xap[hb * 128:(hb + 1) * 128, t0:t0 + tc_n, :])
                    stages.append(xs)
                for ti in range(tc_n):
                    xt = xtp.tile([F, 2 * W2], BF16)
                    for hb in range(2):
                        nc.sync.dma_start_transpose(
                            xt[:, hb * W2:(hb + 1) * W2], stages[hb][:, ti, :])
                    xT.append(xt)

            # ---- main scan ----
            sA_prev = sB_prev = None
            hhA_prev = None

            for t in range(T):
                base = PS_BUF[t % 2]
                obase = PS_BUF[(t + 1) % 2]

                # bias: out[m, n] = b4[n//256, m] over blocks i,f,o,2g
                nc.tensor.matmul(ps[:, base:base + 512], b4[:], e4[:, 0:512],
                                 start=True, stop=False)
                nc.tensor.matmul(ps[:, base + 512:base + 1024], b4[:],
                                 e4[:, 512:1024], start=True, stop=False)
                # xg: 4 chunk matmuls, bf16
                for c in range(4):
                    nc.tensor.matmul(
                        ps[:, base + c * BLK:base + c * BLK + 2 * W2],
                        wih[:, c * H:(c + 1) * H], xT[t][:],
                        start=False, stop=(t == 0))
                # recurrent, wave A (h_half_A(t-1))
                if hhA_prev is not None:
                    for c in range(4):
                        nc.tensor.matmul(
                            ps[:, base + c * BLK:base + c * BLK + W2],
                            whh[:, c * H:(c + 1) * H], hhA_prev[:],
                            start=False, stop=False)

                # packed sigma, wave A: [i|f|o|2g|cd] cols 0:128
                blocks = ps[:, base:base + 1280].rearrange(
                    "p (c n) -> p c n", c=5)
                sA = sigp.tile([128, 5, W2], BF16, tag="sA")
                nc.scalar.activation(sA[:], blocks[:, :, 0:W2], AF.Sigmoid)

                # h_half_B(t-1) = (sig(cd_B) - 0.5) * sigo_B(t-1)
                hhB = None
                if sB_prev is not None:
                    hhB = hhp.tile([H, W2], F32, tag="hhB")
                    nc.vector.scalar_tensor_tensor(
                        hhB[:], sA[:, 4, :], -0.5, sB_prev[:, 2, :],
                        OP.add, OP.mult)

                # wave A cell math
                t2hA = tmpp.tile([H, W2], BF16, tag="t2hA")
                nc.vector.scalar_tensor_tensor(
                    t2hA[:], sA[:, 3, :], -0.5, sA[:, 0, :], OP.add, OP.mult)
                uA = tmpp.tile([H, W2], F32, tag="uA")
                nc.vector.tensor_tensor(
                    uA[:], sA[:, 1, :],
                    ps[:, obase + CD + W2:obase + CD + 2 * W2], OP.mult)
                # cd_A(t) -> current buffer cd block, cols B (read by sigma-op B)
                nc.vector.scalar_tensor_tensor(
                    ps[:, base + CD + W2:base + CD + 2 * W2],
                    t2hA[:], 4.0, uA[:], OP.mult, OP.add)

                # recurrent wave B + wlin for h_B(t-1)
                if hhB is not None:
                    for c in range(4):
                        nc.tensor.matmul(
                            ps[:, base + c * BLK + W2:base + c * BLK + 2 * W2],
                            whh[:, c * H:(c + 1) * H], hhB[:],
                            start=False, stop=True)
                    nc.tensor.matmul(ps[0:1, WLIN_B:WLIN_B + W2],
                                     wl[:, t - 1:t], hhB[:],
                                     start=(t == 1), stop=False)

                # packed sigma, wave B: cols 128:256
                sB = sigp.tile([128, 5, W2], BF16, tag="sB")
                nc.scalar.activation(sB[:], blocks[:, :, W2:2 * W2], AF.Sigmoid)

                # h_half_A(t)
                hhA = hhp.tile([H, W2], F32, tag="hhA")
                nc.vector.scalar_tensor_tensor(
                    hhA[:], sB[:, 4, :], -0.5, sA[:, 2, :], OP.add, OP.mult)

                # wave B cell math
                t2hB = tmpp.tile([H, W2], BF16, tag="t2hB")
                nc.vector.scalar_tensor_tensor(
                    t2hB[:], sB[:, 3, :], -0.5, sB[:, 0, :], OP.add, OP.mult)
                uB = tmpp.tile([H, W2], F32, tag="uB")
                nc.vector.tensor_tensor(
                    uB[:], sB[:, 1, :], ps[:, base + CD:base + CD + W2],
                    OP.mult)
                # cd_B(t) -> other buffer cd block, cols A (read by sigma-op A(t+1))
                nc.vector.scalar_tensor_tensor(
                    ps[:, obase + CD:obase + CD + W2],
                    t2hB[:], 4.0, uB[:], OP.mult, OP.add)

                # wlin for h_A(t)
                nc.tensor.matmul(ps[0:1, WLIN_A:WLIN_A + W2], wl[:, t:t + 1],
                                 hhA[:], start=(t == 0), stop=(t == T - 1))

                if dump:
                    d32 = tmpp.tile([128, 640], F32, tag="d32a")
                    nc.vector.tensor_copy(
                        d32[:], sA[:].rearrange("p c n -> p (c n)"))
                    nc.sync.dma_start(sa_d.ap()[t], d32[:])
                    d32b = tmpp.tile([128, 640], F32, tag="d32b")
                    nc.vector.tensor_copy(
                        d32b[:], sB[:].rearrange("p c n -> p (c n)"))
                    nc.sync.dma_start(sb_d.ap()[t], d32b[:])
                    nc.sync.dma_start(hha_d.ap()[t], hhA[:])
                    if hhB is not None:
                        nc.sync.dma_start(hhb_d.ap()[t - 1], hhB[:])

                sA_prev, sB_prev, hhA_prev = sA, sB, hhA

            # epilogue: h_B(T-1) needs sigma(cd_B(T-1)) (in buffer (T)%2, cols A)
            ebase = PS_BUF[T % 2]
            scd = tmpp.tile([128, W2], BF16, tag="scd")
            nc.scalar.activation(scd[:], ps[:, ebase + CD:ebase + CD + W2],
                                 AF.Sigmoid)
            hhB = hhp.tile([H, W2], F32, tag="hhB")
            nc.vector.scalar_tensor_tensor(
                hhB[:], scd[:], -0.5, sB_prev[:, 2, :], OP.add, OP.mult)
            nc.tensor.matmul(ps[0:1, WLIN_B:WLIN_B + W2], wl[:, T - 1:T],
                             hhB[:], start=False, stop=True)
            if dump:
                nc.sync.dma_start(hhb_d.ap()[T - 1], hhB[:])

            # output
            outsb = constp.tile([1, 2 * W2], F32)
            nc.vector.tensor_copy(outsb[0:1, 0:W2], ps[0:1, WLIN_A:WLIN_A + W2])
            nc.vector.tensor_copy(outsb[0:1, W2:2 * W2],
                                  ps[0:1, WLIN_B:WLIN_B + W2])
            nc.sync.dma_start(out_d.ap().rearrange("(a b) -> a b", a=1),
                              outsb[:])

    nc.compile()
    return nc


_CACHE = {}


def _get_nc(T=T_FULL):
    if T not in _CACHE:
        _CACHE[T] = build(T)
    return _CACHE[T]


def prep_weights(w_ih, w_hh, b_ih, b_hh, w_lin, T=T_FULL):
    """Host-side weight prep. Chunk order [i, f, o, g]; g-chunk pre-scaled x2
    (sigmoid(2g) trick); W_hh and w_lin pre-scaled x2 (h_half absorption)."""
    perm = np.r_[0:H, H:2 * H, 3 * H:4 * H, 2 * H:3 * H]
    gs = np.ones((4 * H, 1), np.float32)
    gs[3 * H:] = 2.0
    bf = ml_dtypes.bfloat16
    whh = np.ascontiguousarray((w_hh[perm] * gs * 2.0).T.astype(np.float32))
    wih = np.ascontiguousarray((w_ih[perm] * gs).T.astype(bf))
    b4 = ((b_ih + b_hh)[perm] * gs[:, 0]).reshape(4, H).astype(bf)
    e4 = np.zeros((4, 1024), bf)
    for c in range(4):
        e4[c, c * 256:(c + 1) * 256] = 1.0
    wl = np.ascontiguousarray((2.0 * w_lin.reshape(T, H)).T.astype(np.float32))
    return whh, wih, b4, e4, wl


def kernel(x, w_ih, w_hh, b_ih, b_hh, w_lin, b_lin):
    x = np.asarray(x, np.float32)
    T = x.shape[1]
    nc = _get_nc(T)
    whh, wih, b4, e4, wl = prep_weights(
        np.asarray(w_ih, np.float32), np.asarray(w_hh, np.float32),
        np.asarray(b_ih, np.float32), np.asarray(b_hh, np.float32),
        np.asarray(w_lin, np.float32), T)
    in_maps = []
    for c in range(NCORES):
        in_maps.append({
            "x": np.ascontiguousarray(x[c * BS:(c + 1) * BS]),
            "whh": whh, "wih": wih, "b4": b4, "e4": e4, "wl": wl,
        })
    res = run_bass_kernel_spmd(nc, in_maps, core_ids=list(range(NCORES)))
    out = np.concatenate([r["out"] for r in res.results])
    return (out + np.float32(b_lin[0])).astype(np.float32)
